# revision 28
# baseline (speedup 1.0000x reference)
"""Trainium2 Bass kernel for nn_BERT_pool_mutil_avr (cosine-attention + ROI pool + conv).

Single fused kernel, 8 cores, each core = (batch, T-half). Host precomputes the
q projection (tiny), folds q into the K weights (wdot) so dot(q,k) falls out of
the x matmuls, bakes ROI bin boundaries + 1/count, and sums the two half-core
outputs at the end. The conv stage runs on-device right after pooling.
"""
import numpy as np
import ml_dtypes

import concourse.mybir as mybir
import concourse.tile as tile
from concourse import bacc, bass_utils

F32 = mybir.dt.float32
F32R = mybir.dt.float32r
BF16 = mybir.dt.bfloat16
I32 = mybir.dt.int32
AF = mybir.ActivationFunctionType
OP = mybir.AluOpType

B, D, T, NROI, H, DK = 4, 1024, 2048, 128, 8, 128
SCALES = [1, 3, 7, 9]
NBT = 20                      # total bins per roi
OFF = [0, 1, 4, 11]           # bin offset of each scale
TH = T // 2                   # tokens per core
KT = D // 128                 # 8 contraction tiles


def _chunks(total, maxc=512):
    nch = -(-total // maxc)
    base = -(-total // nch)
    out, s = [], 0
    while s < total:
        e = min(s + base, total)
        out.append((s, e - s))
        s = e
    return out


def build(npad, has_bv):
    cols = npad * NBT
    cch = _chunks(cols)
    nc = bacc.Bacc("TRN2", target_bir_lowering=False, debug=False, num_devices=8)
    # per-core tensors
    xb = nc.dram_tensor("xb", [D, TH], BF16, kind="ExternalInput").ap()
    wk = nc.dram_tensor("wkT", [D, D], BF16, kind="ExternalInput").ap()
    wv = nc.dram_tensor("wvT", [D, D], BF16, kind="ExternalInput").ap()
    wdot = nc.dram_tensor("wdot", [128, KT * 8], BF16, kind="ExternalInput").ap()
    hmaskd = nc.dram_tensor("hmask", [128, 64], BF16, kind="ExternalInput").ap()
    cc32d = nc.dram_tensor("cc32", [128, 16], F32, kind="ExternalInput").ap()
    ncqd = nc.dram_tensor("ncq", [8, 2], F32, kind="ExternalInput").ap()
    bsd = nc.dram_tensor("bsr", [1, cols], F32R, kind="ExternalInput").ap()
    bed = nc.dram_tensor("ber", [1, cols], F32R, kind="ExternalInput").ap()
    invd = nc.dram_tensor("invr", [1, cols], F32R, kind="ExternalInput").ap()
    onesd = nc.dram_tensor("onesr", [1, 128], F32R, kind="ExternalInput").ap()
    seld = nc.dram_tensor("sel", [8, D], F32R, kind="ExternalInput").ap()
    wts = {nb: nc.dram_tensor(f"wt{nb}", [2 * nb * 128, 256], BF16, kind="ExternalInput").ap()
           for nb in SCALES}
    bvr = nc.dram_tensor("bvr", [128, D], F32, kind="ExternalInput").ap() if has_bv else None
    outd = nc.dram_tensor("out", [npad, D], F32, kind="ExternalOutput").ap()

    with tile.TileContext(nc) as tc:
        with (
            tc.tile_pool(name="const", bufs=1) as cp,
            tc.tile_pool(name="big", bufs=1) as bigp,
            tc.tile_pool(name="k2", bufs=2) as kp,
            tc.tile_pool(name="rows", bufs=1) as rp,
            tc.tile_pool(name="pss", bufs=4, space="PSUM") as pss,
            tc.tile_pool(name="psk", bufs=4, space="PSUM") as psk,
                    ):
            # ---- constants / small inputs
            cc32 = cp.tile([128, 16], F32)      # iota 0, bk 1:9
            nc.sync.dma_start(cc32[:], cc32d[:])
            ncq = cp.tile([8, 2], F32)          # nq2 col 0, cbk col 1
            nc.sync.dma_start(ncq[:], ncqd[:])
            hmask = cp.tile([128, 64], BF16)
            nc.sync.dma_start(hmask[:], hmaskd[:])
            wdot_sb = cp.tile([128, KT * 8], BF16)
            nc.sync.dma_start(wdot_sb[:], wdot[:])
            bs_row = rp.tile([1, cols], F32R)
            be_row = rp.tile([1, cols], F32R)
            inv_row = rp.tile([1, cols], F32R)
            nc.sync.dma_start(bs_row[:], bsd[:])
            nc.sync.dma_start(be_row[:], bed[:])
            nc.sync.dma_start(inv_row[:], invd[:])
            ones_r = cp.tile([1, 128], F32R)
            nc.sync.dma_start(ones_r[:], onesd[:])
            sel_sb = cp.tile([8, D], F32R)
            nc.sync.dma_start(sel_sb[:], seld[:])

            # ---- big DMAs, priority order: x, wk, wv, conv weights
            x_sb = bigp.tile([128, KT, TH], BF16, tag="x")
            x_r = xb.rearrange("(k p) t -> p k t", p=128)
            w_k = bigp.tile([128, KT, D], BF16, tag="wk")
            wk_r = wk.rearrange("(k p) c -> p k c", p=128)
            for k in range(KT):
                nc.sync.dma_start(x_sb[:, k, :], x_r[:, k, :])
            for k in range(KT):
                nc.sync.dma_start(w_k[:, k, :], wk_r[:, k, :])
            w_v = bigp.tile([128, KT, D], BF16, tag="wv")
            wv_r = wv.rearrange("(k p) c -> p k c", p=128)
            for k in range(KT):
                nc.sync.dma_start(w_v[:, k, :], wv_r[:, k, :])
            wt_sb = {}
            for nb in SCALES:
                wt_sb[nb] = bigp.tile([128, 2 * nb, 256], BF16, tag=f"wt{nb}", name=f"wt{nb}_sb")
                nc.sync.dma_start(wt_sb[nb][:], wts[nb].rearrange("(c i p) o -> p (c i) o", p=128, i=nb))
            bv_sb = None
            if has_bv:
                bv_sb = bigp.tile([128, D], F32, tag="bv")
                nc.sync.dma_start(bv_sb[:], bvr[:])

            # ---- broadcast bs/be/inv rows to 128 partitions (PE trick)
            bs_bc = bigp.tile([128, cols], F32, tag="bsbc")
            be_bc = bigp.tile([128, cols], F32, tag="bebc")
            inv_bc = bigp.tile([128, cols], F32, tag="invbc")
            for row, bc in ((bs_row, bs_bc), (be_row, be_bc), (inv_row, inv_bc)):
                for s, w in cch:
                    pb = psk.tile([128, 512], F32, tag="kb")
                    nc.tensor.matmul(pb[:, :w], ones_r[0:1, :], row[0:1, s : s + w], start=True, stop=True)
                    nc.scalar.activation(bc[:, s : s + w], pb[:, :w], AF.Copy)

            # ---- roi masks  mask_sb [128(t), mt, cols] bf16
            thr = cp.tile([128, 4], F32)
            mask_sb = bigp.tile([128, KT, cols], BF16, tag="mask")
            mtmp = rp.tile([128, cols], BF16)
            for mt in range(KT):
                nc.vector.tensor_scalar_add(thr[:, 0:1], cc32[:, 0:1], float(mt * 128) + 0.95)
                nc.vector.tensor_scalar_add(thr[:, 1:2], cc32[:, 0:1], float(mt * 128) + 0.05)
                nc.vector.tensor_scalar(mtmp[:], bs_bc[:], thr[:, 0:1], None, op0=OP.is_lt)
                nc.vector.tensor_scalar(mask_sb[:, mt, :], be_bc[:], thr[:, 1:2], None, op0=OP.is_gt)
                nc.vector.tensor_mul(mask_sb[:, mt, :], mask_sb[:, mt, :], mtmp[:])

            # ---- dot[h, t] via folded weights (streams with x DMA)
            psd = [pss.tile([8, 512], F32, tag="s", name=f"psd{i}") for i in range(2)]
            for k in range(KT):
                for c2 in range(2):
                    nc.tensor.matmul(psd[c2][:], wdot_sb[:, k * 8 : (k + 1) * 8],
                                     x_sb[:, k, c2 * 512 : (c2 + 1) * 512],
                                     start=(k == 0), stop=(k == KT - 1))
            dot_sb = rp.tile([8, TH], F32)
            for c2 in range(2):
                nc.scalar.activation(dot_sb[:, c2 * 512 : (c2 + 1) * 512], psd[c2][:],
                                     AF.Identity, bias=ncq[:, 1:2])

            # ---- K projection -> square -> per-head nk2 (k never materialized)
            psn = [pss.tile([8, 512], F32, tag="s", name=f"psn{i}") for i in range(2)]
            for ct in range(H):
                pk = [psk.tile([128, 512], F32, tag="kb", name=f"pk{ct}_{i}") for i in range(2)]
                for k in range(KT):
                    for c2 in range(2):
                        nc.tensor.matmul(pk[c2][:], w_k[:, k, ct * 128 : (ct + 1) * 128],
                                         x_sb[:, k, c2 * 512 : (c2 + 1) * 512],
                                         start=(k == 0), stop=(k == KT - 1))
                k2t = kp.tile([128, TH], BF16, tag="k2")
                for c2 in range(2):
                    nc.scalar.activation(k2t[:, c2 * 512 : (c2 + 1) * 512], pk[c2][:],
                                         AF.Square, bias=cc32[:, 1 + ct : 2 + ct])
                for c2 in range(2):
                    nc.tensor.matmul(psn[c2][:], hmask[:, ct * 8 : ct * 8 + 8],
                                     k2t[:, c2 * 512 : (c2 + 1) * 512],
                                     start=(ct == 0), stop=(ct == H - 1))

            # ---- p = exp(|dot| / max(nq*nk, 1e-8))
            m_sb = rp.tile([8, TH], F32)
            inv_sb = rp.tile([8, TH], F32)
            pa_sb = rp.tile([8, TH], F32)
            neg_sb = rp.tile([8, TH], F32)
            p_row = rp.tile([8, TH], F32R)
            for c2 in range(2):
                sl = slice(c2 * 512, (c2 + 1) * 512)
                nc.vector.tensor_scalar(m_sb[:, sl], psn[c2][:], ncq[:, 0:1], None, op0=OP.mult)
                nc.vector.tensor_scalar_max(m_sb[:, sl], m_sb[:, sl], 1e-16)
                nc.scalar.activation(inv_sb[:, sl], m_sb[:, sl], AF.Abs_reciprocal_sqrt)
                nc.vector.tensor_mul(pa_sb[:, sl], dot_sb[:, sl], inv_sb[:, sl])
                nc.vector.tensor_scalar_mul(neg_sb[:, sl], pa_sb[:, sl], -1.0)
                nc.vector.tensor_max(pa_sb[:, sl], pa_sb[:, sl], neg_sb[:, sl])
                nc.scalar.activation(p_row[:, sl], pa_sb[:, sl], AF.Exp)

            # ---- V projection + val = p * (v [+ bv])   val_sb [128(t), mt, c] bf16
            # pmul[t, c] = p[h(c), t] via matmul against the selector; deferred one
            # mt-group so the exp chain never stalls the PE
            val_sb = bigp.tile([128, KT, D], BF16, tag="val")
            vtmp = rp.tile([128, 512], F32) if has_bv else None
            pmul_sb = rp.tile([128, 2, 512], F32)
            pvq = []

            def _emit_val(mt, pvs):
                for c2 in range(2):
                    pm = pss.tile([128, 512], F32, tag="s", name=f"pm{mt}_{c2}")
                    nc.tensor.matmul(pm[:], p_row[0:H, mt * 128 : (mt + 1) * 128],
                                     sel_sb[:, c2 * 512 : (c2 + 1) * 512], start=True, stop=True)
                    nc.scalar.activation(pmul_sb[:, c2, :], pm[:], AF.Copy)
                    src_ap = pvs[c2][:]
                    if has_bv:
                        nc.vector.tensor_add(vtmp[:], pvs[c2][:], bv_sb[:, c2 * 512 : (c2 + 1) * 512])
                        src_ap = vtmp[:]
                    nc.vector.tensor_tensor(val_sb[:, mt, c2 * 512 : (c2 + 1) * 512],
                                            src_ap, pmul_sb[:, c2, :], op=OP.mult)

            for mt in range(KT):
                pvs = []
                for c2 in range(2):
                    pv = psk.tile([128, 512], F32, tag="kb", name=f"pv{mt}_{c2}")
                    for k in range(KT):
                        nc.tensor.matmul(pv[:], x_sb[:, k, mt * 128 : (mt + 1) * 128],
                                         w_v[:, k, c2 * 512 : (c2 + 1) * 512],
                                         start=(k == 0), stop=(k == KT - 1))
                    pvs.append(pv)
                pvq.append((mt, pvs))
                if len(pvq) > 1:
                    _emit_val(*pvq.pop(0))
            _emit_val(*pvq.pop(0))

            # ---- pooling (P scaled by 1/count) + conv, interleaved per scale
            P_sb = bigp.tile([128, H, cols], BF16, tag="P")
            P_r = P_sb.rearrange("p c (n i) -> p c n i", i=NBT)
            out_sb = rp.tile([128, D], F32)
            for j in (3, 2, 1, 0):
                nb = SCALES[j]
                for ctl in range(2):
                    ct = 2 * j + ctl
                    for s, w in cch:
                        pp = pss.tile([128, 512], F32, tag="s")
                        for mt in range(KT):
                            nc.tensor.matmul(pp[:, :w], val_sb[:, mt, ct * 128 : (ct + 1) * 128],
                                             mask_sb[:, mt, s : s + w],
                                             start=(mt == 0), stop=(mt == KT - 1))
                        nc.vector.tensor_tensor(P_sb[:, ct, s : s + w], pp[:, :w],
                                                inv_bc[:, s : s + w], op=OP.mult)
                pc = psk.tile([128, 256], F32, tag="kb")
                mms = [(ctl, i) for ctl in range(2) for i in range(nb)]
                for idx, (ctl, i) in enumerate(mms):
                    ct = 2 * j + ctl
                    nc.tensor.matmul(pc[:npad, :], P_r[:, ct, :, OFF[j] + i],
                                     wt_sb[nb][:, ctl * nb + i, :],
                                     start=(idx == 0), stop=(idx == len(mms) - 1))
                nc.scalar.activation(out_sb[:npad, j * 256 : (j + 1) * 256], pc[:npad, :], AF.Copy)
                nc.sync.dma_start(outd[:, j * 256 : (j + 1) * 256], out_sb[:npad, j * 256 : (j + 1) * 256])

    nc.compile()
    return nc


TRACE = False
LAST_EXEC_NS = -1
LAST_TRACES = []


def _run(nc, in_maps, label):
    global LAST_EXEC_NS, LAST_TRACES
    if not TRACE:
        return bass_utils.run_bass_kernel_spmd(nc, in_maps, core_ids=list(range(8)))
    r = bass_utils.run_bass_kernel_spmd(
        nc, in_maps, core_ids=list(range(8)), trace=True,
        trace_kwargs={"title": label},
    )
    if r.exec_time_ns is not None:
        if LAST_EXEC_NS < 0:
            LAST_EXEC_NS = 0
        LAST_EXEC_NS += r.exec_time_ns
    if r.instructions_and_trace is not None:
        LAST_TRACES.append((label, r.instructions_and_trace[1], r.exec_time_ns))
    return r


def kernel(**inputs):
    iv = np.asarray(inputs["input_vectors"], np.float32)
    cls = np.asarray(inputs["clstoken_scales"], np.float32)
    rois = np.asarray(inputs["rois"], np.int32)
    Wq = np.asarray(inputs["Wq"], np.float64)
    Wk = np.asarray(inputs["Wk"], np.float64)
    bq = np.asarray(inputs["bq"], np.float64)
    bk = np.asarray(inputs["bk"], np.float64)
    bv = np.asarray(inputs["bv"], np.float32)
    has_bv = bool(np.any(bv))

    wvT16 = np.asarray(inputs["Wv"], np.float32).T.astype(ml_dtypes.bfloat16)
    wkT16 = np.asarray(inputs["Wk"], np.float32).T.astype(ml_dtypes.bfloat16)

    # host: q projection + per-head fold
    q = cls.astype(np.float64) @ Wq.T + bq                    # [B, D]
    qh = q.reshape(B, H, DK)
    nq2 = (qh * qh).sum(-1)                                   # [B, H]
    cbk = (qh * bk.reshape(H, DK)).sum(-1)                    # [B, H]
    # wdot[b, d, h] = sum_dk Wk[h*DK+dk, d] * q[b, h*DK+dk]
    wdot = np.einsum("hkd,bhk->bdh", Wk.reshape(H, DK, D), qh)  # [B, D, H]

    # host: group rois by batch, bin boundaries, counts
    order = np.argsort(rois[:, 0], kind="stable")
    rs = rois[order]
    starts, counts = [], []
    for b in range(B):
        idx = np.nonzero(rs[:, 0] == b)[0]
        starts.append(int(idx[0]) if len(idx) else 0)
        counts.append(len(idx))
    npad = max(max(counts), 1)
    cols = npad * NBT
    padded = []
    for b in range(B):
        arr = np.zeros((npad, 3), np.int64)
        arr[:, 2] = 16
        if counts[b]:
            arr[: counts[b]] = rs[starts[b] : starts[b] + counts[b]]
        padded.append(arr)

    fl = np.zeros(NBT, np.int64)
    fh = np.zeros(NBT, np.int64)
    for j, nb in enumerate(SCALES):
        for i in range(nb):
            fl[OFF[j] + i] = i
            fh[OFF[j] + i] = i + 1
    nbv = np.zeros(NBT, np.int64)
    for j, nb in enumerate(SCALES):
        nbv[OFF[j] : OFF[j] + nb] = nb

    bs_b, be_b, inv_b = [], [], []
    for b in range(B):
        s, e = padded[b][:, 1:2], padded[b][:, 2:3]
        L = e - s
        bs = s + (fl[None, :] * L) // nbv[None, :]
        be = s - (-(fh[None, :] * L)) // nbv[None, :]
        cnt = np.maximum(be - bs, 1)
        bs_b.append(bs.reshape(-1).astype(np.float32))
        be_b.append(be.reshape(-1).astype(np.float32))
        inv_b.append((1.0 / cnt.reshape(-1)).astype(np.float32))

    hmask = np.zeros((128, 64), ml_dtypes.bfloat16)
    for ct in range(H):
        hmask[:, ct * 8 + ct] = 1.0
    cc32 = np.zeros((128, 16), np.float32)
    cc32[:, 0] = np.arange(128)
    cc32[:, 1:9] = bk.reshape(H, DK).T.astype(np.float32)

    wt_in = {}
    for j, nb in enumerate(SCALES):
        cw = np.asarray(inputs[f"conv_w{nb}"], np.float32)          # [o, c, i]
        a = cw.transpose(1, 2, 0).reshape(2, 128, nb, 256)          # [ctl, p, i, o]
        wt_in[nb] = np.ascontiguousarray(
            a.transpose(0, 2, 1, 3).reshape(2 * nb * 128, 256)).astype(ml_dtypes.bfloat16)
    cbias = np.concatenate([np.asarray(inputs[f"conv_b{nb}"], np.float32) for nb in SCALES])

    selc = np.zeros((8, D), np.float32)
    for h in range(H):
        selc[h, h * DK : (h + 1) * DK] = 1.0

    nc = build(npad, has_bv)
    in_maps = []
    for core in range(8):
        b, half = core // 2, core % 2
        m = {
            "xb": np.ascontiguousarray(iv[b, :, half * TH : (half + 1) * TH]).astype(ml_dtypes.bfloat16),
            "wkT": wkT16, "wvT": wvT16,
            "wdot": np.ascontiguousarray(
                wdot[b].reshape(KT, 128, H).transpose(1, 0, 2).reshape(128, KT * 8)
            ).astype(ml_dtypes.bfloat16),
            "hmask": hmask, "cc32": cc32,
            "ncq": np.stack([nq2[b], cbk[b]], axis=1).astype(np.float32),
            "bsr": (bs_b[b] - half * TH)[None, :],
            "ber": (be_b[b] - half * TH)[None, :],
            "invr": inv_b[b][None, :],
            "onesr": np.ones((1, 128), np.float32),
            "sel": selc,
        }
        for nb in SCALES:
            m[f"wt{nb}"] = wt_in[nb]
        if has_bv:
            m["bvr"] = np.ascontiguousarray(np.broadcast_to(bv, (128, D)))
        in_maps.append(m)
    r = _run(nc, in_maps, "k1")
    final = np.empty((NROI, D), np.float32)
    stacked = np.empty((len(rs), D), np.float32)
    for b in range(B):
        if counts[b]:
            sl = slice(starts[b], starts[b] + counts[b])
            stacked[sl] = (r.results[2 * b]["out"][: counts[b]]
                           + r.results[2 * b + 1]["out"][: counts[b]] + cbias)
    final[order] = stacked
    return final


# revision 29
# speedup vs baseline: 1.2091x; 1.2091x over previous
"""Trainium2 Bass kernel for nn_BERT_pool_mutil_avr (cosine-attention + ROI pool + conv).

Single fused kernel, 8 cores, each core = (batch, T-half). Host precomputes the
q projection (tiny), folds q into the K weights (wdot) so dot(q,k) falls out of
the x matmuls, bakes ROI bin boundaries + 1/count, and sums the two half-core
outputs at the end. The conv stage runs on-device right after pooling.
"""
import numpy as np
import ml_dtypes

import concourse.mybir as mybir
import concourse.tile as tile
from concourse import bacc, bass_utils
from concourse.masks import make_identity

F32 = mybir.dt.float32
F32R = mybir.dt.float32r
BF16 = mybir.dt.bfloat16
I32 = mybir.dt.int32
AF = mybir.ActivationFunctionType
OP = mybir.AluOpType

B, D, T, NROI, H, DK = 4, 1024, 2048, 128, 8, 128
SCALES = [1, 3, 7, 9]
NBT = 20                      # total bins per roi
OFF = [0, 1, 4, 11]           # bin offset of each scale
TH = T // 2                   # tokens per core
KT = D // 128                 # 8 contraction tiles


def _chunks(total, maxc=512):
    nch = -(-total // maxc)
    base = -(-total // nch)
    out, s = [], 0
    while s < total:
        e = min(s + base, total)
        out.append((s, e - s))
        s = e
    return out


def build(npad, has_bv):
    cols = npad * NBT
    cch = _chunks(cols)
    nc = bacc.Bacc("TRN2", target_bir_lowering=False, debug=False, num_devices=8)
    # per-core tensors
    xb = nc.dram_tensor("xb", [D, TH], BF16, kind="ExternalInput").ap()
    wk = nc.dram_tensor("wkT", [D, D], BF16, kind="ExternalInput").ap()
    wv = nc.dram_tensor("wvT", [D, D], BF16, kind="ExternalInput").ap()
    wdot = nc.dram_tensor("wdot", [128, KT * 8], BF16, kind="ExternalInput").ap()
    hmaskd = nc.dram_tensor("hmask", [128, 64], BF16, kind="ExternalInput").ap()
    cc32d = nc.dram_tensor("cc32", [128, 16], F32, kind="ExternalInput").ap()
    ncqd = nc.dram_tensor("ncq", [8, 2], F32, kind="ExternalInput").ap()
    bsd = nc.dram_tensor("bsr", [1, cols], F32R, kind="ExternalInput").ap()
    bed = nc.dram_tensor("ber", [1, cols], F32R, kind="ExternalInput").ap()
    invd = nc.dram_tensor("invr", [1, cols], F32R, kind="ExternalInput").ap()
    onesd = nc.dram_tensor("onesr", [1, 128], F32R, kind="ExternalInput").ap()
    wts = {nb: nc.dram_tensor(f"wt{nb}", [2 * nb * 128, 256], BF16, kind="ExternalInput").ap()
           for nb in SCALES}
    bvr = nc.dram_tensor("bvr", [128, D], F32, kind="ExternalInput").ap() if has_bv else None
    outd = nc.dram_tensor("out", [npad, D], F32, kind="ExternalOutput").ap()

    with tile.TileContext(nc) as tc:
        with (
            tc.tile_pool(name="const", bufs=1) as cp,
            tc.tile_pool(name="big", bufs=1) as bigp,
            tc.tile_pool(name="k2", bufs=2) as kp,
            tc.tile_pool(name="rows", bufs=1) as rp,
            tc.tile_pool(name="pss", bufs=4, space="PSUM") as pss,
            tc.tile_pool(name="psk", bufs=4, space="PSUM") as psk,
                    ):
            # ---- constants / small inputs
            cc32 = cp.tile([128, 16], F32)      # iota 0, bk 1:9
            nc.sync.dma_start(cc32[:], cc32d[:])
            ncq = cp.tile([8, 2], F32)          # nq2 col 0, cbk col 1
            nc.sync.dma_start(ncq[:], ncqd[:])
            hmask = cp.tile([128, 64], BF16)
            nc.sync.dma_start(hmask[:], hmaskd[:])
            wdot_sb = cp.tile([128, KT * 8], BF16)
            nc.sync.dma_start(wdot_sb[:], wdot[:])
            bs_row = rp.tile([1, cols], F32R)
            be_row = rp.tile([1, cols], F32R)
            inv_row = rp.tile([1, cols], F32R)
            nc.sync.dma_start(bs_row[:], bsd[:])
            nc.sync.dma_start(be_row[:], bed[:])
            nc.sync.dma_start(inv_row[:], invd[:])
            ones_r = cp.tile([1, 128], F32R)
            nc.sync.dma_start(ones_r[:], onesd[:])
            ident = cp.tile([128, 128], F32)
            make_identity(nc, ident[:])

            # ---- big DMAs, priority order: x, wk, wv, conv weights
            x_sb = bigp.tile([128, KT, TH], BF16, tag="x")
            x_r = xb.rearrange("(k p) t -> p k t", p=128)
            w_k = bigp.tile([128, KT, D], BF16, tag="wk")
            wk_r = wk.rearrange("(k p) c -> p k c", p=128)
            for k in range(KT):
                nc.sync.dma_start(x_sb[:, k, :], x_r[:, k, :])
            for k in range(KT):
                nc.sync.dma_start(w_k[:, k, :], wk_r[:, k, :])
            w_v = bigp.tile([128, KT, D], BF16, tag="wv")
            wv_r = wv.rearrange("(k p) c -> p k c", p=128)
            for k in range(KT):
                nc.sync.dma_start(w_v[:, k, :], wv_r[:, k, :])
            wt_sb = {}
            for nb in SCALES:
                wt_sb[nb] = bigp.tile([128, 2 * nb, 256], BF16, tag=f"wt{nb}", name=f"wt{nb}_sb")
                nc.sync.dma_start(wt_sb[nb][:], wts[nb].rearrange("(c i p) o -> p (c i) o", p=128, i=nb))
            bv_sb = None
            if has_bv:
                bv_sb = bigp.tile([128, D], F32, tag="bv")
                nc.sync.dma_start(bv_sb[:], bvr[:])

            # ---- broadcast bs/be/inv rows to 128 partitions (PE trick)
            bs_bc = bigp.tile([128, cols], F32, tag="bsbc")
            be_bc = bigp.tile([128, cols], F32, tag="bebc")
            inv_bc = bigp.tile([128, cols], F32, tag="invbc")
            for row, bc in ((bs_row, bs_bc), (be_row, be_bc), (inv_row, inv_bc)):
                for s, w in cch:
                    pb = psk.tile([128, 512], F32, tag="kb")
                    nc.tensor.matmul(pb[:, :w], ones_r[0:1, :], row[0:1, s : s + w], start=True, stop=True)
                    nc.scalar.activation(bc[:, s : s + w], pb[:, :w], AF.Copy)

            # ---- roi masks  mask_sb [128(t), mt, cols] bf16
            thr = cp.tile([128, 4], F32)
            mask_sb = bigp.tile([128, KT, cols], BF16, tag="mask")
            mtmp = rp.tile([128, cols], BF16)
            for mt in range(KT):
                nc.vector.tensor_scalar_add(thr[:, 0:1], cc32[:, 0:1], float(mt * 128) + 0.95)
                nc.vector.tensor_scalar_add(thr[:, 1:2], cc32[:, 0:1], float(mt * 128) + 0.05)
                nc.vector.tensor_scalar(mtmp[:], bs_bc[:], thr[:, 0:1], None, op0=OP.is_lt)
                nc.vector.tensor_scalar(mask_sb[:, mt, :], be_bc[:], thr[:, 1:2], None, op0=OP.is_gt)
                nc.vector.tensor_mul(mask_sb[:, mt, :], mask_sb[:, mt, :], mtmp[:])

            # ---- dot[h, t] via folded weights (streams with x DMA)
            psd = [pss.tile([8, 512], F32, tag="s", name=f"psd{i}") for i in range(2)]
            for k in range(KT):
                for c2 in range(2):
                    nc.tensor.matmul(psd[c2][:], wdot_sb[:, k * 8 : (k + 1) * 8],
                                     x_sb[:, k, c2 * 512 : (c2 + 1) * 512],
                                     start=(k == 0), stop=(k == KT - 1))
            dot_sb = rp.tile([8, TH], F32)
            for c2 in range(2):
                nc.scalar.activation(dot_sb[:, c2 * 512 : (c2 + 1) * 512], psd[c2][:],
                                     AF.Identity, bias=ncq[:, 1:2])

            # ---- K projection -> square -> per-head nk2 (k never materialized)
            psn = [pss.tile([8, 512], F32, tag="s", name=f"psn{i}") for i in range(2)]
            for ct in range(H):
                pk = [psk.tile([128, 512], F32, tag="kb", name=f"pk{ct}_{i}") for i in range(2)]
                for k in range(KT):
                    for c2 in range(2):
                        nc.tensor.matmul(pk[c2][:], w_k[:, k, ct * 128 : (ct + 1) * 128],
                                         x_sb[:, k, c2 * 512 : (c2 + 1) * 512],
                                         start=(k == 0), stop=(k == KT - 1))
                k2t = kp.tile([128, TH], BF16, tag="k2")
                for c2 in range(2):
                    nc.scalar.activation(k2t[:, c2 * 512 : (c2 + 1) * 512], pk[c2][:],
                                         AF.Square, bias=cc32[:, 1 + ct : 2 + ct])
                for c2 in range(2):
                    nc.tensor.matmul(psn[c2][:], hmask[:, ct * 8 : ct * 8 + 8],
                                     k2t[:, c2 * 512 : (c2 + 1) * 512],
                                     start=(ct == 0), stop=(ct == H - 1))

            # ---- p = exp(|dot| / max(nq*nk, 1e-8))
            m_sb = rp.tile([8, TH], F32)
            inv_sb = rp.tile([8, TH], F32)
            pa_sb = rp.tile([8, TH], F32)
            neg_sb = rp.tile([8, TH], F32)
            p_row = rp.tile([8, TH], F32)
            for c2 in range(2):
                sl = slice(c2 * 512, (c2 + 1) * 512)
                nc.vector.tensor_scalar(m_sb[:, sl], psn[c2][:], ncq[:, 0:1], None, op0=OP.mult)
                nc.vector.tensor_scalar_max(m_sb[:, sl], m_sb[:, sl], 1e-16)
                nc.scalar.activation(inv_sb[:, sl], m_sb[:, sl], AF.Abs_reciprocal_sqrt)
                nc.vector.tensor_mul(pa_sb[:, sl], dot_sb[:, sl], inv_sb[:, sl])
                nc.vector.tensor_scalar_mul(neg_sb[:, sl], pa_sb[:, sl], -1.0)
                nc.vector.tensor_max(pa_sb[:, sl], pa_sb[:, sl], neg_sb[:, sl])
                nc.scalar.activation(p_row[:, sl], pa_sb[:, sl], AF.Exp)

            # ---- V projection + val = p * (v [+ bv])   val_sb [128(t), mt, c] bf16
            val_sb = bigp.tile([128, KT, D], BF16, tag="val")
            vtmp = rp.tile([128, 512], F32) if has_bv else None
            pT_sb = rp.tile([128, 64], F32)   # col mt*8+h = p[h, mt*128+part]
            pvq = []

            def _emit_val(mt, pvs):
                for c2 in range(2):
                    src_ap = pvs[c2][:]
                    if has_bv:
                        nc.vector.tensor_add(vtmp[:], pvs[c2][:], bv_sb[:, c2 * 512 : (c2 + 1) * 512])
                        src_ap = vtmp[:]
                    for hl in range(4):
                        h = c2 * 4 + hl
                        nc.vector.tensor_scalar(
                            val_sb[:, mt, c2 * 512 + hl * 128 : c2 * 512 + (hl + 1) * 128],
                            src_ap[:, hl * 128 : (hl + 1) * 128],
                            pT_sb[:, mt * 8 + h : mt * 8 + h + 1], None, op0=OP.mult)

            for mt in range(KT):
                pvs = []
                for c2 in range(2):
                    pv = psk.tile([128, 512], F32, tag="kb", name=f"pv{mt}_{c2}")
                    for k in range(KT):
                        nc.tensor.matmul(pv[:], x_sb[:, k, mt * 128 : (mt + 1) * 128],
                                         w_v[:, k, c2 * 512 : (c2 + 1) * 512],
                                         start=(k == 0), stop=(k == KT - 1))
                    pvs.append(pv)
                pvq.append((mt, pvs))
                if mt == 1:
                    for tmt in range(KT):
                        tp = pss.tile([128, 8], F32, tag="s", name=f"tp{tmt}")
                        nc.tensor.transpose(tp[:], p_row[0:H, tmt * 128 : (tmt + 1) * 128], ident[0:H, 0:H])
                        nc.scalar.activation(pT_sb[:, tmt * 8 : (tmt + 1) * 8], tp[:], AF.Copy)
                if len(pvq) > 1:
                    _emit_val(*pvq.pop(0))
            _emit_val(*pvq.pop(0))

            # ---- pooling (P scaled by 1/count) + conv, interleaved per scale
            P_sb = bigp.tile([128, H, cols], BF16, tag="P")
            P_r = P_sb.rearrange("p c (n i) -> p c n i", i=NBT)
            out_sb = rp.tile([128, D], F32)
            for j in (3, 2, 1, 0):
                nb = SCALES[j]
                for ctl in range(2):
                    ct = 2 * j + ctl
                    for s, w in cch:
                        pp = pss.tile([128, 512], F32, tag="s")
                        for mt in range(KT):
                            nc.tensor.matmul(pp[:, :w], val_sb[:, mt, ct * 128 : (ct + 1) * 128],
                                             mask_sb[:, mt, s : s + w],
                                             start=(mt == 0), stop=(mt == KT - 1))
                        nc.vector.tensor_tensor(P_sb[:, ct, s : s + w], pp[:, :w],
                                                inv_bc[:, s : s + w], op=OP.mult)
                pc = psk.tile([128, 256], F32, tag="kb")
                mms = [(ctl, i) for ctl in range(2) for i in range(nb)]
                for idx, (ctl, i) in enumerate(mms):
                    ct = 2 * j + ctl
                    nc.tensor.matmul(pc[:npad, :], P_r[:, ct, :, OFF[j] + i],
                                     wt_sb[nb][:, ctl * nb + i, :],
                                     start=(idx == 0), stop=(idx == len(mms) - 1))
                nc.scalar.activation(out_sb[:npad, j * 256 : (j + 1) * 256], pc[:npad, :], AF.Copy)
                nc.sync.dma_start(outd[:, j * 256 : (j + 1) * 256], out_sb[:npad, j * 256 : (j + 1) * 256])

    nc.compile()
    return nc


TRACE = False
LAST_EXEC_NS = -1
LAST_TRACES = []


def _run(nc, in_maps, label):
    global LAST_EXEC_NS, LAST_TRACES
    if not TRACE:
        return bass_utils.run_bass_kernel_spmd(nc, in_maps, core_ids=list(range(8)))
    r = bass_utils.run_bass_kernel_spmd(
        nc, in_maps, core_ids=list(range(8)), trace=True,
        trace_kwargs={"title": label},
    )
    if r.exec_time_ns is not None:
        if LAST_EXEC_NS < 0:
            LAST_EXEC_NS = 0
        LAST_EXEC_NS += r.exec_time_ns
    if r.instructions_and_trace is not None:
        LAST_TRACES.append((label, r.instructions_and_trace[1], r.exec_time_ns))
    return r


def kernel(**inputs):
    iv = np.asarray(inputs["input_vectors"], np.float32)
    cls = np.asarray(inputs["clstoken_scales"], np.float32)
    rois = np.asarray(inputs["rois"], np.int32)
    Wq = np.asarray(inputs["Wq"], np.float64)
    Wk = np.asarray(inputs["Wk"], np.float64)
    bq = np.asarray(inputs["bq"], np.float64)
    bk = np.asarray(inputs["bk"], np.float64)
    bv = np.asarray(inputs["bv"], np.float32)
    has_bv = bool(np.any(bv))

    wvT16 = np.asarray(inputs["Wv"], np.float32).T.astype(ml_dtypes.bfloat16)
    wkT16 = np.asarray(inputs["Wk"], np.float32).T.astype(ml_dtypes.bfloat16)

    # host: q projection + per-head fold
    q = cls.astype(np.float64) @ Wq.T + bq                    # [B, D]
    qh = q.reshape(B, H, DK)
    nq2 = (qh * qh).sum(-1)                                   # [B, H]
    cbk = (qh * bk.reshape(H, DK)).sum(-1)                    # [B, H]
    # wdot[b, d, h] = sum_dk Wk[h*DK+dk, d] * q[b, h*DK+dk]
    wdot = np.einsum("hkd,bhk->bdh", Wk.reshape(H, DK, D), qh)  # [B, D, H]

    # host: group rois by batch, bin boundaries, counts
    order = np.argsort(rois[:, 0], kind="stable")
    rs = rois[order]
    starts, counts = [], []
    for b in range(B):
        idx = np.nonzero(rs[:, 0] == b)[0]
        starts.append(int(idx[0]) if len(idx) else 0)
        counts.append(len(idx))
    npad = max(max(counts), 1)
    cols = npad * NBT
    padded = []
    for b in range(B):
        arr = np.zeros((npad, 3), np.int64)
        arr[:, 2] = 16
        if counts[b]:
            arr[: counts[b]] = rs[starts[b] : starts[b] + counts[b]]
        padded.append(arr)

    fl = np.zeros(NBT, np.int64)
    fh = np.zeros(NBT, np.int64)
    for j, nb in enumerate(SCALES):
        for i in range(nb):
            fl[OFF[j] + i] = i
            fh[OFF[j] + i] = i + 1
    nbv = np.zeros(NBT, np.int64)
    for j, nb in enumerate(SCALES):
        nbv[OFF[j] : OFF[j] + nb] = nb

    bs_b, be_b, inv_b = [], [], []
    for b in range(B):
        s, e = padded[b][:, 1:2], padded[b][:, 2:3]
        L = e - s
        bs = s + (fl[None, :] * L) // nbv[None, :]
        be = s - (-(fh[None, :] * L)) // nbv[None, :]
        cnt = np.maximum(be - bs, 1)
        bs_b.append(bs.reshape(-1).astype(np.float32))
        be_b.append(be.reshape(-1).astype(np.float32))
        inv_b.append((1.0 / cnt.reshape(-1)).astype(np.float32))

    hmask = np.zeros((128, 64), ml_dtypes.bfloat16)
    for ct in range(H):
        hmask[:, ct * 8 + ct] = 1.0
    cc32 = np.zeros((128, 16), np.float32)
    cc32[:, 0] = np.arange(128)
    cc32[:, 1:9] = bk.reshape(H, DK).T.astype(np.float32)

    wt_in = {}
    for j, nb in enumerate(SCALES):
        cw = np.asarray(inputs[f"conv_w{nb}"], np.float32)          # [o, c, i]
        a = cw.transpose(1, 2, 0).reshape(2, 128, nb, 256)          # [ctl, p, i, o]
        wt_in[nb] = np.ascontiguousarray(
            a.transpose(0, 2, 1, 3).reshape(2 * nb * 128, 256)).astype(ml_dtypes.bfloat16)
    cbias = np.concatenate([np.asarray(inputs[f"conv_b{nb}"], np.float32) for nb in SCALES])

    nc = build(npad, has_bv)
    in_maps = []
    for core in range(8):
        b, half = core // 2, core % 2
        m = {
            "xb": np.ascontiguousarray(iv[b, :, half * TH : (half + 1) * TH]).astype(ml_dtypes.bfloat16),
            "wkT": wkT16, "wvT": wvT16,
            "wdot": np.ascontiguousarray(
                wdot[b].reshape(KT, 128, H).transpose(1, 0, 2).reshape(128, KT * 8)
            ).astype(ml_dtypes.bfloat16),
            "hmask": hmask, "cc32": cc32,
            "ncq": np.stack([nq2[b], cbk[b]], axis=1).astype(np.float32),
            "bsr": (bs_b[b] - half * TH)[None, :],
            "ber": (be_b[b] - half * TH)[None, :],
            "invr": inv_b[b][None, :],
            "onesr": np.ones((1, 128), np.float32),
        }
        for nb in SCALES:
            m[f"wt{nb}"] = wt_in[nb]
        if has_bv:
            m["bvr"] = np.ascontiguousarray(np.broadcast_to(bv, (128, D)))
        in_maps.append(m)
    r = _run(nc, in_maps, "k1")
    final = np.empty((NROI, D), np.float32)
    stacked = np.empty((len(rs), D), np.float32)
    for b in range(B):
        if counts[b]:
            sl = slice(starts[b], starts[b] + counts[b])
            stacked[sl] = (r.results[2 * b]["out"][: counts[b]]
                           + r.results[2 * b + 1]["out"][: counts[b]] + cbias)
    final[order] = stacked
    return final
